# revision 1
# baseline (speedup 1.0000x reference)
"""Multi-head attention (B=4, N=2048, C=1024, H=16) on 8 TRN2 NeuronCores.

Sharding: core = 2*b + half handles batch b, heads half*8 .. half*8+7.
Each core computes QKV for its 8 heads, full attention for them, and a
partial projection (its 512 rows of W_proj). Host sums the two partials
per batch and adds the bias.

All matmul operands are fp16 (1 cycle/row on the PE vs 2 for f32r, same
~11-bit mantissa); accumulation stays fp32 in PSUM. The host pre-casts
weights/x to fp16 and pre-transposes x so x^T tiles DMA in contiguously.

On-chip layout is "transposed": Q^T/K^T [d, n] come straight out of the
QKV matmuls (lhsT = W chunk, rhs = x^T), scores are computed as
S^T[m, n] so that exp(S^T) = P^T is directly the moving operand of the
AV matmul (V chunk stationary). exp is shifted by a constant bias (it
cancels in softmax) to keep P in fp16 range. Row sums of P ride along as
a 65th stationary column of ones; the normalization (reciprocal,
partition broadcast via K=1 matmul, multiply) happens lazily in SBUF so
PSUM banks recycle immediately.
"""

import functools
from contextlib import ExitStack

import numpy as np

import concourse.bass as bass
import concourse.tile as tile
from concourse import bacc, mybir
from concourse.bass_utils import run_bass_kernel_spmd

F32 = mybir.dt.float32
F32R = mybir.dt.float32r
F16 = mybir.dt.float16
AF = mybir.ActivationFunctionType

B, N, C = 4, 2048, 1024
H, D = 16, 64
P = 128
NCORES = 8
HPC = 8            # heads per core
PAIRS = HPC // 2   # 4
DCORE = HPC * D    # 512 attention columns per core
SCALE = float(H) ** -0.5  # 0.25 (faithful to reference: num_heads**-0.5)
EXP_BIAS = -5.0    # exp(scale*s + bias): cancels in softmax, keeps fp16 range
NB = N // 512      # 4 n blocks
NT = N // P        # 16 tiles of 128
CT = C // P        # 8 contraction chunks
VW = D + 1         # V columns per head incl. the ones column (row sums)
MBLK = HPC * VW    # 520 v_sb columns per m-tile

LAST_RESULT = None  # BassKernelResults of the most recent run (for test.py)


def _kernel_body(tc, out_d, xt_d, wq_d, wk_d, wv_d, wp_d):
    nc = tc.nc
    with ExitStack() as ctx:
        const = ctx.enter_context(tc.tile_pool(name="const", bufs=1))
        ones_f = const.tile([P, P], F32)
        nc.vector.memset(ones_f, 1.0)
        ones_bc = const.tile([P, 64], F16)
        nc.vector.tensor_copy(ones_bc, ones_f[:, 0:64])
        ebias = const.tile([P, 1], F32)
        nc.vector.memset(ebias, EXP_BIAS)

        # attT: pair p occupies cols [p*N, (p+1)*N); partitions = 2 heads x 64
        attT_pool = ctx.enter_context(tc.tile_pool(name="attT", bufs=1))
        attT = attT_pool.tile([P, PAIRS * N], F16)

        # PSUM: mm 2 + s 2x2 + av 2 (tags avA/avB) = 8 banks
        ps_mm = ctx.enter_context(tc.tile_pool(name="ps_mm", bufs=2, space="PSUM"))
        ps_s = ctx.enter_context(tc.tile_pool(name="ps_s", bufs=2, space="PSUM"))
        ps_av = ctx.enter_context(tc.tile_pool(name="ps_av", bufs=1, space="PSUM"))

        with ExitStack() as mid:
            # x^T: c-chunk j at cols [j*N, (j+1)*N); DMA'd directly (host
            # pre-transposed x)
            xt_pool = mid.enter_context(tc.tile_pool(name="xt", bufs=1))
            xt = xt_pool.tile([P, CT * N], F16)
            for j in range(CT):
                nc.sync.dma_start(out=xt[:, j * N:(j + 1) * N],
                                  in_=xt_d[j * P:(j + 1) * P, :])
            # V: m-tile m at cols [m*MBLK, ...); head hl at
            # [m*MBLK + hl*VW, +D], then a ones column (for row sums)
            v_pool = mid.enter_context(tc.tile_pool(name="v", bufs=1))
            v_sb = v_pool.tile([P, NT * MBLK], F16)
            ones_cols = v_sb.rearrange("q (g k) -> q g k", k=VW)[:, :, D:VW]
            nc.vector.tensor_copy(
                ones_cols, ones_f.rearrange("q (g k) -> q g k", k=1))

            # ---- Phase B1: V for all 8 heads ----
            with tc.tile_pool(name="wv", bufs=1) as wv_pool:
                wv_sb = wv_pool.tile([P, CT * DCORE], F16)
                for cc in range(CT):
                    nc.sync.dma_start(
                        out=wv_sb[:, cc * DCORE:(cc + 1) * DCORE],
                        in_=wv_d[cc * P:(cc + 1) * P, :])
                for m in range(NT):
                    psv = ps_mm.tile([P, DCORE], F32, tag="mm")
                    for cc in range(CT):
                        nc.tensor.matmul(
                            psv,
                            xt[:, cc * N + m * P: cc * N + (m + 1) * P],
                            wv_sb[:, cc * DCORE:(cc + 1) * DCORE],
                            start=(cc == 0), stop=(cc == CT - 1))
                    nc.vector.tensor_copy(
                        v_sb[:, m * MBLK:(m + 1) * MBLK].rearrange(
                            "q (h k) -> q h k", k=VW)[:, :, 0:D],
                        psv.rearrange("q (h k) -> q h k", k=D))

            # ---- Phases B2 + C, interleaved per head pair ----
            qt_pool = mid.enter_context(tc.tile_pool(name="qt", bufs=2))
            kt_pool = mid.enter_context(tc.tile_pool(name="kt", bufs=2))
            wqk_pool = mid.enter_context(tc.tile_pool(name="wqk", bufs=3))
            pt_pool = mid.enter_context(tc.tile_pool(name="pt", bufs=4))
            rb_pool = mid.enter_context(tc.tile_pool(name="rb", bufs=3))
            rc_pool = mid.enter_context(tc.tile_pool(name="rc", bufs=3))

            for p in range(PAIRS):
                # B2: Q^T and K^T for the pair (partitions: head 2p dims
                # 0-63, head 2p+1 dims 64-127)
                qt = qt_pool.tile([P, N], F16, tag="qt")
                kt = kt_pool.tile([P, N], F16, tag="kt")
                for w_d, dst in ((wq_d, qt), (wk_d, kt)):
                    wt = wqk_pool.tile([P, CT * P], F16, tag="w")
                    nc.sync.dma_start(
                        out=wt.rearrange("q (cc f) -> q cc f", cc=CT),
                        in_=w_d[:, p * P:(p + 1) * P].rearrange(
                            "(cc q) f -> q cc f", q=P))
                    for nb in range(NB):
                        psq = ps_mm.tile([P, 512], F32, tag="mm")
                        for cc in range(CT):
                            nc.tensor.matmul(
                                psq,
                                wt[:, cc * P:(cc + 1) * P],
                                xt[:, cc * N + nb * 512: cc * N + nb * 512 + 512],
                                start=(cc == 0), stop=(cc == CT - 1))
                        nc.vector.tensor_copy(dst[:, nb * 512:(nb + 1) * 512], psq)

                # C: attention for the pair
                for nb in range(NB):
                    nsl = slice(nb * 512, nb * 512 + 512)
                    osl = slice(p * N + nb * 512, p * N + nb * 512 + 512)
                    ps_av_a = ps_av.tile([P, 512], F32, tag="avA")
                    ps_av_b = ps_av.tile([P, 512], F32, tag="avB")
                    for m in range(NT):
                        first = (m == 0)
                        last = (m == NT - 1)
                        ps_s_t = ps_s.tile([P, 1024], F32, tag="s")
                        # scores^T chunk [m-tile, n-block]; heads row-tiled
                        nc.tensor.matmul(
                            ps_s_t[:, 0:512],
                            kt[0:64, m * P:(m + 1) * P],
                            qt[0:64, nsl],
                            start=True, stop=True)
                        nc.tensor.matmul(
                            ps_s_t[:, 512:1024],
                            kt[64:128, m * P:(m + 1) * P],
                            qt[64:128, nsl],
                            start=True, stop=True)
                        pt = pt_pool.tile([P, 1024], F16, tag="pt")
                        nc.scalar.activation(pt, ps_s_t, AF.Exp,
                                             scale=SCALE, bias=ebias)
                        # AV with fused row-sums: lhsT = [V_h | 1] (M = 65);
                        # partition 64 accumulates the softmax denominators
                        vbase = m * MBLK + 2 * p * VW
                        nc.tensor.matmul(
                            ps_av_a[0:VW, :],
                            v_sb[:, vbase: vbase + VW],
                            pt[:, 0:512],
                            start=first, stop=last, skip_group_check=True)
                        nc.tensor.matmul(
                            ps_av_b[0:VW, :],
                            v_sb[:, vbase + VW: vbase + 2 * VW],
                            pt[:, 512:1024],
                            start=first, stop=last, skip_group_check=True)
                    # evict eagerly (free the PSUM banks), normalize lazily
                    rc = rc_pool.tile([P, 1024], F16, tag="rc")
                    nc.vector.tensor_copy(rc[64:65, 0:512], ps_av_a[D:VW, :])
                    nc.vector.tensor_copy(rc[64:65, 512:1024], ps_av_b[D:VW, :])
                    nc.vector.tensor_copy(attT[0:64, osl], ps_av_a[0:64, :])
                    tmb = rb_pool.tile([64, 512], F16, tag="tmb")
                    nc.vector.tensor_copy(tmb, ps_av_b[0:64, :])
                    # lazy: fp16 reciprocal + K=1 broadcast matmul + mul
                    with nc.allow_low_precision(
                            reason="softmax recip rounding is benign"):
                        nc.vector.reciprocal(rc[64:65, :], rc[64:65, :])
                    ps_rb_a = ps_mm.tile([P, 512], F32, tag="mm")
                    nc.tensor.matmul(
                        ps_rb_a[0:64, :], ones_bc[64:65, :],
                        rc[64:65, 0:512],
                        start=True, stop=True, tile_position=(64, 0),
                        skip_group_check=True)
                    ps_rb_b = ps_mm.tile([P, 512], F32, tag="mm")
                    nc.tensor.matmul(
                        ps_rb_b[0:64, :], ones_bc[64:65, :],
                        rc[64:65, 512:1024],
                        start=True, stop=True, tile_position=(64, 0),
                        skip_group_check=True)
                    rb = rb_pool.tile([64, 1024], F32, tag="rb")
                    nc.vector.tensor_copy(rb[:, 0:512], ps_rb_a[0:64, :])
                    nc.vector.tensor_copy(rb[:, 512:1024], ps_rb_b[0:64, :])
                    nc.vector.tensor_mul(attT[0:64, osl],
                                         attT[0:64, osl], rb[:, 0:512])
                    nc.vector.tensor_mul(tmb, tmb, rb[:, 512:1024])
                    # head B's rows sit at partitions 0-63; shift to 64-127
                    nc.sync.dma_start(out=attT[64:128, osl], in_=tmb)

        # ---- Phase D: partial projection out = attT.T @ wp ----
        with tc.tile_pool(name="wp", bufs=1) as wp_pool, \
                tc.tile_pool(name="stage", bufs=3) as stage_pool:
            wp_sb = wp_pool.tile([P, PAIRS * C], F16)
            for dc in range(PAIRS):
                nc.sync.dma_start(out=wp_sb[:, dc * C:(dc + 1) * C],
                                  in_=wp_d[dc * P:(dc + 1) * P, :])
            for i in range(NT):
                for co in range(2):
                    psp = ps_mm.tile([P, 512], F32, tag="mm")
                    for dc in range(PAIRS):
                        nc.tensor.matmul(
                            psp,
                            attT[:, dc * N + i * P: dc * N + (i + 1) * P],
                            wp_sb[:, dc * C + co * 512: dc * C + co * 512 + 512],
                            start=(dc == 0), stop=(dc == PAIRS - 1))
                    st = stage_pool.tile([P, 512], F32, tag="st")
                    nc.vector.tensor_copy(st, psp)
                    nc.sync.dma_start(
                        out=out_d[i * P:(i + 1) * P, co * 512: co * 512 + 512],
                        in_=st)


@functools.lru_cache(maxsize=1)
def build_nc():
    nc = bacc.Bacc("TRN2", target_bir_lowering=False, debug=False)
    xt_d = nc.dram_tensor("xt_local", [C, N], F16, kind="ExternalInput").ap()
    wq_d = nc.dram_tensor("wq", [C, DCORE], F16, kind="ExternalInput").ap()
    wk_d = nc.dram_tensor("wk", [C, DCORE], F16, kind="ExternalInput").ap()
    wv_d = nc.dram_tensor("wv", [C, DCORE], F16, kind="ExternalInput").ap()
    wp_d = nc.dram_tensor("wp", [DCORE, C], F16, kind="ExternalInput").ap()
    out_d = nc.dram_tensor("out_partial", [N, C], F32, kind="ExternalOutput").ap()
    with tile.TileContext(nc) as tc:
        _kernel_body(tc, out_d, xt_d, wq_d, wk_d, wv_d, wp_d)
    nc.compile()
    return nc


def make_in_maps(x, W_qkv, W_proj):
    in_maps = []
    for core in range(NCORES):
        b, half = core // 2, core % 2
        h0 = half * HPC
        in_maps.append({
            "xt_local": np.ascontiguousarray(x[b].T.astype(np.float16)),
            "wq": np.ascontiguousarray(
                W_qkv[:, 0 * C + h0 * D: 0 * C + h0 * D + DCORE].astype(np.float16)),
            "wk": np.ascontiguousarray(
                W_qkv[:, 1 * C + h0 * D: 1 * C + h0 * D + DCORE].astype(np.float16)),
            "wv": np.ascontiguousarray(
                W_qkv[:, 2 * C + h0 * D: 2 * C + h0 * D + DCORE].astype(np.float16)),
            "wp": np.ascontiguousarray(
                W_proj[h0 * D: h0 * D + DCORE, :].astype(np.float16)),
        })
    return in_maps


def kernel(x, W_qkv, W_proj, b_proj, trace=False):
    x = np.asarray(x, dtype=np.float32)
    W_qkv = np.asarray(W_qkv, dtype=np.float32)
    W_proj = np.asarray(W_proj, dtype=np.float32)
    b_proj = np.asarray(b_proj, dtype=np.float32)

    nc = build_nc()
    in_maps = make_in_maps(x, W_qkv, W_proj)

    global LAST_RESULT
    res = run_bass_kernel_spmd(nc, in_maps, list(range(NCORES)), trace=trace)
    LAST_RESULT = res

    out = np.empty((B, N, C), dtype=np.float32)
    for b in range(B):
        out[b] = (res.results[2 * b]["out_partial"]
                  + res.results[2 * b + 1]["out_partial"]
                  + b_proj[None, :])
    return out



# revision 5
# speedup vs baseline: 1.2131x; 1.2131x over previous
"""Multi-head attention (B=4, N=2048, C=1024, H=16) on 8 TRN2 NeuronCores.

Sharding: core = 2*b + half handles batch b, heads half*8 .. half*8+7.
Each core computes QKV for its 8 heads, full attention for them, and a
partial projection (its 512 rows of W_proj). Host sums the two partials
per batch and adds the bias.

v2 schedule: the scalar engine's exp stream (256 x [128,1024], ~284us)
is the critical resource; everything else is arranged so neither it nor
the PE ever hits a head-of-line block:
  - softmax normalization is PE-free and off the critical path: the
    denominators ride the AV matmuls as a 65th stationary column; the
    sums row is evicted, spread across 128 partitions via a DRAM
    round-trip, reciprocal'd wide (~0.2us instead of 6.5us on one
    partition), broadcast back with a stride-0 DMA and multiplied in
    on DVE.  These DMAs ride the gpsimd (SWDGE) queue so the sync
    queue never blocks on them.
  - V for head pairs 0-1 and Q/K for pair 0 are computed up front,
    overlapping the x^T DMA; V for pairs 2-3, Q/K for later pairs and
    the first 12 projection tiles are emitted as small fill units
    inside the attention blocks, where the PE has slack while the exp
    stream runs.
  - PSUM evictions go to the scalar engine only where it is idle
    (V boot, projection tail), otherwise to DVE.

All matmul operands are fp16 (1 cycle/row on the PE), accumulation
fp32 in PSUM. Host pre-casts weights/x and pre-transposes x; output
partials return as fp16 and are summed on the host in fp32.
"""

import functools
from contextlib import ExitStack

import numpy as np

import concourse.bass as bass
import concourse.tile as tile
from concourse import bacc, mybir
from concourse.bass_utils import run_bass_kernel_spmd

F32 = mybir.dt.float32
F16 = mybir.dt.float16
AF = mybir.ActivationFunctionType

B, N, C = 4, 2048, 1024
H, D = 16, 64
P = 128
NCORES = 8
HPC = 8            # heads per core
PAIRS = HPC // 2   # 4
DCORE = HPC * D    # 512 attention columns per core
SCALE = float(H) ** -0.5  # 0.25 (faithful to reference: num_heads**-0.5)
EXP_BIAS = -5.0    # exp(scale*s + bias): cancels in softmax, keeps fp16 range
NB = N // 512      # 4 query blocks
NT = N // P        # 16 key tiles of 128
CT = C // P        # 8 contraction chunks
VW = D + 1         # V columns per head incl. the ones column (row sums)
MBLK = HPC * VW    # 520 v_sb columns per m-tile
HB = DCORE // 2    # 256 V columns per half (head pairs 0-1 / 2-3)

LAST_RESULT = None  # BassKernelResults of the most recent run (for test.py)


def _kernel_body(tc, out_d, xt_d, wq_d, wk_d, wv_d, wp_d):
    nc = tc.nc
    with ExitStack() as ctx:
        const = ctx.enter_context(tc.tile_pool(name="const", bufs=1))
        ones_f = const.tile([P, P], F32)
        nc.vector.memset(ones_f, 1.0)
        ebias = const.tile([P, 1], F32)
        nc.vector.memset(ebias, EXP_BIAS)

        # attT: pair p occupies cols [p*N, (p+1)*N); partitions = 2 heads x 64
        attT_pool = ctx.enter_context(tc.tile_pool(name="attT", bufs=1))
        attT = attT_pool.tile([P, PAIRS * N], F16)
        xt_pool = ctx.enter_context(tc.tile_pool(name="xt", bufs=1))
        xt = xt_pool.tile([P, CT * N], F16)
        v_pool = ctx.enter_context(tc.tile_pool(name="v", bufs=1))
        v_sb = v_pool.tile([P, NT * MBLK], F16)
        wv_pool = ctx.enter_context(tc.tile_pool(name="wv", bufs=1))
        wv_sb = wv_pool.tile([P, CT * DCORE], F16)
        wp_pool = ctx.enter_context(tc.tile_pool(name="wp", bufs=1))
        wp_sb = wp_pool.tile([P, PAIRS * C], F16)

        qt_pool = ctx.enter_context(tc.tile_pool(name="qt", bufs=2))
        kt_pool = ctx.enter_context(tc.tile_pool(name="kt", bufs=2))
        wqk_pool = ctx.enter_context(tc.tile_pool(name="wqk", bufs=4))
        pt_pool = ctx.enter_context(tc.tile_pool(name="pt", bufs=6))
        tmb_pool = ctx.enter_context(tc.tile_pool(name="tmb", bufs=3))
        sums_pool = ctx.enter_context(tc.tile_pool(name="sums", bufs=2))
        spread_pool = ctx.enter_context(tc.tile_pool(name="spread", bufs=2))
        rb_pool = ctx.enter_context(tc.tile_pool(name="rb", bufs=2))
        stage_pool = ctx.enter_context(tc.tile_pool(name="stage", bufs=3))
        dram_pool = ctx.enter_context(
            tc.tile_pool(name="dscr", bufs=3, space="DRAM"))

        ps_mm = ctx.enter_context(tc.tile_pool(name="ps_mm", bufs=2, space="PSUM"))

        qt_tiles = [None] * PAIRS
        kt_tiles = [None] * PAIRS
        wt_tiles = [None] * PAIRS

        def emit_wqk_dma(p):
            tiles = []
            for w_d in (wq_d, wk_d):
                wt = wqk_pool.tile([P, CT * P], F16, tag="w")
                nc.sync.dma_start(
                    out=wt.rearrange("q (cc f) -> q cc f", cc=CT),
                    in_=w_d[:, p * P:(p + 1) * P].rearrange(
                        "(cc q) f -> q cc f", q=P))
                tiles.append(wt)
            wt_tiles[p] = tiles

        # ---- DMA order sets the boot critical path ----
        emit_wqk_dma(0)
        for cc in range(CT):  # wv chunks for head pairs 0-1
            nc.sync.dma_start(out=wv_sb[:, cc * DCORE:cc * DCORE + HB],
                              in_=wv_d[cc * P:(cc + 1) * P, 0:HB])
        for j in range(CT):
            nc.sync.dma_start(out=xt[:, j * N:(j + 1) * N],
                              in_=xt_d[j * P:(j + 1) * P, :])
        for cc in range(CT):  # wv chunks for head pairs 2-3
            nc.sync.dma_start(out=wv_sb[:, cc * DCORE + HB:(cc + 1) * DCORE],
                              in_=wv_d[cc * P:(cc + 1) * P, HB:DCORE])

        # ones columns of v_sb (fused softmax row sums)
        ones_cols = v_sb.rearrange("q (g k) -> q g k", k=VW)[:, :, D:VW]
        nc.vector.tensor_copy(
            ones_cols, ones_f.rearrange("q (g k) -> q g k", k=1))

        def emit_v_tile(half, m, pool, evict_vector):
            c0 = half * HB
            psv = pool.tile([P, HB], F32,
                            tag="bv" if pool is not ps_mm else "mm")
            for cc in range(CT):
                nc.tensor.matmul(
                    psv,
                    xt[:, cc * N + m * P: cc * N + (m + 1) * P],
                    wv_sb[:, cc * DCORE + c0: cc * DCORE + c0 + HB],
                    start=(cc == 0), stop=(cc == CT - 1))
            base = m * MBLK + 4 * half * VW
            dst = v_sb[:, base: base + 4 * VW].rearrange(
                "q (h k) -> q h k", k=VW)[:, :, 0:D]
            src = psv.rearrange("q (h k) -> q h k", k=D)
            if evict_vector:
                nc.vector.tensor_copy(dst, src)
            else:
                nc.scalar.copy(dst, src)

        def emit_qk_block(p, which, nb):
            # which: 0 = q, 1 = k
            wt = wt_tiles[p][which]
            dst = (qt_tiles if which == 0 else kt_tiles)[p]
            psq = ps_mm.tile([P, 512], F32, tag="mm")
            for cc in range(CT):
                nc.tensor.matmul(
                    psq,
                    wt[:, cc * P:(cc + 1) * P],
                    xt[:, cc * N + nb * 512: cc * N + nb * 512 + 512],
                    start=(cc == 0), stop=(cc == CT - 1))
            nc.vector.tensor_copy(dst[:, nb * 512:(nb + 1) * 512], psq)

        def emit_proj(i, evict_vector):
            for co in range(2):
                psp = ps_mm.tile([P, 512], F32, tag="mm")
                for dc in range(PAIRS):
                    nc.tensor.matmul(
                        psp,
                        attT[:, dc * N + i * P: dc * N + (i + 1) * P],
                        wp_sb[:, dc * C + co * 512: dc * C + co * 512 + 512],
                        start=(dc == 0), stop=(dc == PAIRS - 1))
                st = stage_pool.tile([P, 512], F16, tag="st")
                if evict_vector:
                    nc.vector.tensor_copy(st, psp)
                else:
                    nc.scalar.copy(st, psp)
                nc.sync.dma_start(
                    out=out_d[i * P:(i + 1) * P, co * 512: co * 512 + 512],
                    in_=st)

        # ---- boot: V(pairs 0-1) on 6 PSUM banks; K^T(p0); Q^T(p0,b0) ----
        qt_tiles[0] = qt_pool.tile([P, N], F16, tag="qt", name="qt0")
        kt_tiles[0] = kt_pool.tile([P, N], F16, tag="kt", name="kt0")
        with tc.tile_pool(name="ps_boot", bufs=6, space="PSUM") as ps_boot:
            for m in range(NT):
                emit_v_tile(0, m, ps_boot, evict_vector=False)
            for nb in range(NB):
                emit_qk_block(0, 1, nb)
            emit_qk_block(0, 0, 0)

        ps_s = ctx.enter_context(tc.tile_pool(name="ps_s", bufs=2, space="PSUM"))
        ps_av = ctx.enter_context(tc.tile_pool(name="ps_av", bufs=1, space="PSUM"))

        # ---- fill units: small chunks of PE work dropped into the slack
        # of the exp-bound attention blocks ----
        def qtf(p, nb):
            return lambda: emit_qk_block(p, 0, nb)

        def ktf(p, nb):
            return lambda: emit_qk_block(p, 1, nb)

        def vf(m):
            return lambda: emit_v_tile(1, m, ps_mm, evict_vector=True)

        def projf(i):
            return lambda: emit_proj(i, evict_vector=True)

        def newpair(p):
            def f():
                qt_tiles[p] = qt_pool.tile([P, N], F16, tag="qt", name=f"qt{p}")
                kt_tiles[p] = kt_pool.tile([P, N], F16, tag="kt", name=f"kt{p}")
                emit_wqk_dma(p)
            return f

        def wpdma():
            def f():
                for dc in range(PAIRS):
                    nc.sync.dma_start(out=wp_sb[:, dc * C:(dc + 1) * C],
                                      in_=wp_d[dc * P:(dc + 1) * P, :])
            return f

        FILLS = {
            (0, 0): [qtf(0, 1), newpair(1), wpdma(), vf(0), vf(1)],
            (0, 1): [qtf(0, 2), ktf(1, 0), vf(2), vf(3)],
            (0, 2): [qtf(0, 3), ktf(1, 1), vf(4), vf(5)],
            (0, 3): [ktf(1, 2), ktf(1, 3), qtf(1, 0), vf(6), vf(7)],
            (1, 0): [newpair(2), qtf(1, 1), vf(8), vf(9)],
            (1, 1): [qtf(1, 2), ktf(2, 0), vf(10), vf(11)],
            (1, 2): [qtf(1, 3), ktf(2, 1), vf(12), vf(13)],
            (1, 3): [ktf(2, 2), ktf(2, 3), qtf(2, 0), vf(14), vf(15)],
            (2, 0): [newpair(3), qtf(2, 1)],
            (2, 1): [qtf(2, 2), ktf(3, 0)],
            (2, 2): [qtf(2, 3), ktf(3, 1)],
            (2, 3): [ktf(3, 2), ktf(3, 3), qtf(3, 0)],
            (3, 0): [qtf(3, 1)],
            (3, 1): [qtf(3, 2), projf(0), projf(1), projf(2), projf(3)],
            (3, 2): [qtf(3, 3), projf(4), projf(5), projf(6), projf(7)],
            (3, 3): [projf(8), projf(9), projf(10), projf(11)],
        }
        FILL_AT = (2, 5, 8, 11, 14)

        def emit_c_block(p, nb):
            qt_t, kt_t = qt_tiles[p], kt_tiles[p]
            fills = list(FILLS.get((p, nb), []))
            nsl = slice(nb * 512, nb * 512 + 512)
            osl = slice(p * N + nb * 512, p * N + nb * 512 + 512)
            ps_av_t = ps_av.tile([P, 1024], F32, tag="av")
            for m in range(NT):
                if m in FILL_AT and fills:
                    fills.pop(0)()
                first, last = (m == 0), (m == NT - 1)
                ps_s_t = ps_s.tile([P, 1024], F32, tag="s")
                nc.tensor.matmul(
                    ps_s_t[:, 0:512],
                    kt_t[0:64, m * P:(m + 1) * P], qt_t[0:64, nsl],
                    start=True, stop=True)
                nc.tensor.matmul(
                    ps_s_t[:, 512:1024],
                    kt_t[64:128, m * P:(m + 1) * P], qt_t[64:128, nsl],
                    start=True, stop=True)
                pt = pt_pool.tile([P, 1024], F16, tag="pt")
                nc.scalar.activation(pt, ps_s_t, AF.Exp,
                                     scale=SCALE, bias=ebias)
                vbase = m * MBLK + 2 * p * VW
                nc.tensor.matmul(
                    ps_av_t[0:VW, 0:512],
                    v_sb[:, vbase: vbase + VW],
                    pt[:, 0:512],
                    start=first, stop=last, skip_group_check=True)
                nc.tensor.matmul(
                    ps_av_t[0:VW, 512:1024],
                    v_sb[:, vbase + VW: vbase + 2 * VW],
                    pt[:, 512:1024],
                    start=first, stop=last, skip_group_check=True)
            while fills:
                fills.pop(0)()
            # evict: head A rows -> attT, head B rows -> tmb, sums -> DRAM
            nc.vector.tensor_copy(attT[0:64, osl], ps_av_t[0:64, 0:512])
            tmb = tmb_pool.tile([64, 512], F16, tag="tmb")
            nc.vector.tensor_copy(tmb, ps_av_t[0:64, 512:1024])
            srow = sums_pool.tile([P, 1024], F32, tag="srow")
            nc.vector.tensor_copy(srow[64:65, :], ps_av_t[64:65, :])
            dsum = dram_pool.tile([1024], F32, tag="dsum")
            nc.gpsimd.dma_start(out=dsum.rearrange("(a b) -> a b", a=1),
                                in_=srow[64:65, :])
            # normalization (PE-free): spread sums across 128 partitions via
            # DRAM, wide reciprocal, stride-0 broadcast back, multiply in.
            spread = spread_pool.tile([P, 8], F32, tag="spf")
            nc.gpsimd.dma_start(out=spread,
                                in_=dsum.rearrange("(q f) -> q f", q=P))
            spreadr = spread_pool.tile([P, 8], F16, tag="sph")
            with nc.allow_low_precision(reason="softmax recip rounding"):
                nc.vector.reciprocal(spreadr, spread)
            drec = dram_pool.tile([1024], F16, tag="drec")
            nc.gpsimd.dma_start(out=drec.rearrange("(q f) -> q f", q=P),
                                in_=spreadr)
            rb = rb_pool.tile([64, 1024], F16, tag="rb")
            nc.gpsimd.dma_start(
                out=rb,
                in_=drec.rearrange("(a b) -> a b", a=1).broadcast_to([64, 1024]))
            nc.vector.tensor_mul(attT[0:64, osl], attT[0:64, osl], rb[:, 0:512])
            nc.vector.tensor_mul(tmb, tmb, rb[:, 512:1024])
            nc.sync.dma_start(out=attT[64:128, osl], in_=tmb)

        for p in range(PAIRS):
            for nb in range(NB):
                emit_c_block(p, nb)

        # ---- tail: projection of the last query block ----
        for i in range(4 * (NB - 1), 4 * NB):
            emit_proj(i, evict_vector=False)


@functools.lru_cache(maxsize=1)
def build_nc():
    nc = bacc.Bacc("TRN2", target_bir_lowering=False, debug=False)
    xt_d = nc.dram_tensor("xt_local", [C, N], F16, kind="ExternalInput").ap()
    wq_d = nc.dram_tensor("wq", [C, DCORE], F16, kind="ExternalInput").ap()
    wk_d = nc.dram_tensor("wk", [C, DCORE], F16, kind="ExternalInput").ap()
    wv_d = nc.dram_tensor("wv", [C, DCORE], F16, kind="ExternalInput").ap()
    wp_d = nc.dram_tensor("wp", [DCORE, C], F16, kind="ExternalInput").ap()
    out_d = nc.dram_tensor("out_partial", [N, C], F16, kind="ExternalOutput").ap()
    with tile.TileContext(nc) as tc:
        _kernel_body(tc, out_d, xt_d, wq_d, wk_d, wv_d, wp_d)
    nc.compile()
    return nc


def make_in_maps(x, W_qkv, W_proj):
    in_maps = []
    for core in range(NCORES):
        b, half = core // 2, core % 2
        h0 = half * HPC
        in_maps.append({
            "xt_local": np.ascontiguousarray(x[b].T.astype(np.float16)),
            "wq": np.ascontiguousarray(
                W_qkv[:, 0 * C + h0 * D: 0 * C + h0 * D + DCORE].astype(np.float16)),
            "wk": np.ascontiguousarray(
                W_qkv[:, 1 * C + h0 * D: 1 * C + h0 * D + DCORE].astype(np.float16)),
            "wv": np.ascontiguousarray(
                W_qkv[:, 2 * C + h0 * D: 2 * C + h0 * D + DCORE].astype(np.float16)),
            "wp": np.ascontiguousarray(
                W_proj[h0 * D: h0 * D + DCORE, :].astype(np.float16)),
        })
    return in_maps


def kernel(x, W_qkv, W_proj, b_proj, trace=False):
    x = np.asarray(x, dtype=np.float32)
    W_qkv = np.asarray(W_qkv, dtype=np.float32)
    W_proj = np.asarray(W_proj, dtype=np.float32)
    b_proj = np.asarray(b_proj, dtype=np.float32)

    nc = build_nc()
    in_maps = make_in_maps(x, W_qkv, W_proj)

    global LAST_RESULT
    res = run_bass_kernel_spmd(nc, in_maps, list(range(NCORES)), trace=trace)
    LAST_RESULT = res

    out = np.empty((B, N, C), dtype=np.float32)
    for b in range(B):
        out[b] = (res.results[2 * b]["out_partial"].astype(np.float32)
                  + res.results[2 * b + 1]["out_partial"].astype(np.float32)
                  + b_proj[None, :])
    return out


# revision 7
# speedup vs baseline: 1.4229x; 1.1730x over previous
"""Multi-head attention (B=4, N=2048, C=1024, H=16) on 8 TRN2 NeuronCores.

Sharding: core = 2*b + half handles batch b, heads half*8 .. half*8+7.
Each core computes QKV for its 8 heads, full attention for them, and a
partial projection (its 512 rows of W_proj). Host sums the two partials
per batch and adds the bias.

v3 schedule: the scalar engine's exp stream (256 x [128,1024]) is the
critical resource; everything else is arranged so neither it nor the
PE ever hits a head-of-line block:
  - x^T is staged in DRAM as 32 contiguous (query-block, c-chunk)
    blocks and DMA'd in dependency-chained groups, so K^T/Q^T for the
    first query block (and the exp stream) start after ~1 MB of
    traffic instead of after the full 4 MB.
  - softmax normalization is PE-free and off the critical path: the
    denominators ride the AV matmuls as a 65th stationary column; the
    sums row is evicted, spread across 128 partitions via a DRAM
    round-trip, reciprocal'd wide (~0.2us instead of 6.5us on one
    partition), broadcast back with a stride-0 DMA and multiplied in
    on DVE.  These DMAs ride the gpsimd (SWDGE) queue so the sync
    queue never blocks on them.
  - V (pairs 0-1 beyond the first tiles), V (pairs 2-3), Q/K for later
    pairs and the first 12 projection tiles are emitted as small fill
    units inside the attention blocks, sized to the PE slack there.
  - PSUM evictions go to the scalar engine only where it is idle
    (boot, projection tail), otherwise to DVE.

All matmul operands are fp16 (1 cycle/row on the PE), accumulation
fp32 in PSUM. Host pre-casts weights/x and pre-transposes x; output
partials return as fp16 and are summed on the host in fp32.
"""

import functools
from contextlib import ExitStack

import numpy as np

import concourse.bass as bass
import concourse.tile as tile
from concourse.tile import add_dep_helper
from concourse import bacc, mybir
from concourse.bass_utils import run_bass_kernel_spmd

F32 = mybir.dt.float32
F16 = mybir.dt.float16
AF = mybir.ActivationFunctionType

B, N, C = 4, 2048, 1024
H, D = 16, 64
P = 128
NCORES = 8
HPC = 8            # heads per core
PAIRS = HPC // 2   # 4
DCORE = HPC * D    # 512 attention columns per core
SCALE = float(H) ** -0.5  # 0.25 (faithful to reference: num_heads**-0.5)
EXP_BIAS = -5.0    # exp(scale*s + bias): cancels in softmax, keeps fp16 range
NB = N // 512      # 4 query blocks
NT = N // P        # 16 key tiles of 128
CT = C // P        # 8 contraction chunks
VW = D + 1         # V columns per head incl. the ones column (row sums)
MBLK = HPC * VW    # 520 v_sb columns per m-tile
HB = DCORE // 2    # 256 V columns per half (head pairs 0-1 / 2-3)

LAST_RESULT = None  # BassKernelResults of the most recent run (for test.py)


def _kernel_body(tc, out_d, xtb_d, wq_d, wk_d, wv_d, wp_d):
    nc = tc.nc
    with ExitStack() as ctx:
        const = ctx.enter_context(tc.tile_pool(name="const", bufs=1))
        ones_f = const.tile([P, P], F32)
        nc.vector.memset(ones_f, 1.0)
        ebias = const.tile([P, 1], F32)
        nc.vector.memset(ebias, EXP_BIAS)

        # attT: pair p occupies cols [p*N, (p+1)*N); partitions = 2 heads x 64
        attT_pool = ctx.enter_context(tc.tile_pool(name="attT", bufs=1))
        attT = attT_pool.tile([P, PAIRS * N], F16)
        xt_pool = ctx.enter_context(tc.tile_pool(name="xt", bufs=1))
        xt = xt_pool.tile([P, CT * N], F16)
        v_pool = ctx.enter_context(tc.tile_pool(name="v", bufs=1))
        v_sb = v_pool.tile([P, NT * MBLK], F16)
        wv_pool = ctx.enter_context(tc.tile_pool(name="wv", bufs=1))
        wv_sb = wv_pool.tile([P, CT * DCORE], F16)
        wp_pool = ctx.enter_context(tc.tile_pool(name="wp", bufs=1))
        wp_sb = wp_pool.tile([P, PAIRS * C], F16)

        qt_pool = ctx.enter_context(tc.tile_pool(name="qt", bufs=2))
        kt_pool = ctx.enter_context(tc.tile_pool(name="kt", bufs=2))
        wqk_pool = ctx.enter_context(tc.tile_pool(name="wqk", bufs=4))
        pt_pool = ctx.enter_context(tc.tile_pool(name="pt", bufs=6))
        tmb_pool = ctx.enter_context(tc.tile_pool(name="tmb", bufs=3))
        sums_pool = ctx.enter_context(tc.tile_pool(name="sums", bufs=2))
        spread_pool = ctx.enter_context(tc.tile_pool(name="spread", bufs=2))
        rb_pool = ctx.enter_context(tc.tile_pool(name="rb", bufs=2))
        stage_pool = ctx.enter_context(tc.tile_pool(name="stage", bufs=3))
        dram_pool = ctx.enter_context(
            tc.tile_pool(name="dscr", bufs=3, space="DRAM"))

        ps_mm = ctx.enter_context(tc.tile_pool(name="ps_mm", bufs=2, space="PSUM"))

        qt_tiles = [None] * PAIRS
        kt_tiles = [None] * PAIRS
        wt_tiles = [None] * PAIRS

        def emit_wqk_dma(p):
            tiles = []
            for w_d in (wq_d, wk_d):
                wt = wqk_pool.tile([P, CT * P], F16, tag="w")
                nc.sync.dma_start(
                    out=wt.rearrange("q (cc f) -> q cc f", cc=CT),
                    in_=w_d[:, p * P:(p + 1) * P].rearrange(
                        "(cc q) f -> q cc f", q=P))
                tiles.append(wt)
            wt_tiles[p] = tiles

        def emit_wv_dma(half):
            c0 = half * HB
            insts = []
            for cc in range(CT):
                insts.append(nc.sync.dma_start(
                    out=wv_sb[:, cc * DCORE + c0: cc * DCORE + c0 + HB],
                    in_=wv_d[cc * P:(cc + 1) * P, c0:c0 + HB]))
            return insts

        # ---- boot DMAs, dependency-chained so the first query block's
        # x^T (1 MB) and wv(pairs 0-1) land before the rest of x^T ----
        emit_wqk_dma(0)
        xt_last = {}

        def emit_xt_group(nb, after=None):
            last = None
            for cc in range(CT):
                ins = nc.sync.dma_start(
                    out=xt[:, cc * N + nb * 512: cc * N + nb * 512 + 512],
                    in_=xtb_d[(nb * CT + cc) * P:(nb * CT + cc + 1) * P, :])
                if after is not None:
                    add_dep_helper(ins.ins, after.ins, sync=True,
                                   reason="boot DMA pacing")
                last = ins
            xt_last[nb] = last
            return last

        g_a = emit_xt_group(0)
        wv01 = emit_wv_dma(0)
        for ins in wv01:
            add_dep_helper(ins.ins, g_a.ins, sync=True,
                           reason="boot DMA pacing")
        g_c = emit_xt_group(1, after=wv01[-1])
        g_d = emit_xt_group(2, after=g_c)
        emit_xt_group(3, after=g_d)

        # ones columns of v_sb (fused softmax row sums)
        ones_cols = v_sb.rearrange("q (g k) -> q g k", k=VW)[:, :, D:VW]
        nc.vector.tensor_copy(
            ones_cols, ones_f.rearrange("q (g k) -> q g k", k=1))

        def emit_v_tile(half, m, pool, evict_vector):
            c0 = half * HB
            psv = pool.tile([P, HB], F32,
                            tag="mm" if pool is ps_mm else "bv")
            for cc in range(CT):
                nc.tensor.matmul(
                    psv,
                    xt[:, cc * N + m * P: cc * N + (m + 1) * P],
                    wv_sb[:, cc * DCORE + c0: cc * DCORE + c0 + HB],
                    start=(cc == 0), stop=(cc == CT - 1))
            base = m * MBLK + 4 * half * VW
            dst = v_sb[:, base: base + 4 * VW].rearrange(
                "q (h k) -> q h k", k=VW)[:, :, 0:D]
            src = psv.rearrange("q (h k) -> q h k", k=D)
            if evict_vector:
                nc.vector.tensor_copy(dst, src)
            else:
                nc.scalar.copy(dst, src)

        def emit_qk_block(p, which, nb):
            # which: 0 = q, 1 = k
            wt = wt_tiles[p][which]
            dst = (qt_tiles if which == 0 else kt_tiles)[p]
            psq = ps_mm.tile([P, 512], F32, tag="mm")
            for cc in range(CT):
                nc.tensor.matmul(
                    psq,
                    wt[:, cc * P:(cc + 1) * P],
                    xt[:, cc * N + nb * 512: cc * N + nb * 512 + 512],
                    start=(cc == 0), stop=(cc == CT - 1))
            nc.vector.tensor_copy(dst[:, nb * 512:(nb + 1) * 512], psq)

        def emit_proj(i, evict_vector):
            for co in range(2):
                psp = ps_mm.tile([P, 512], F32, tag="mm")
                for dc in range(PAIRS):
                    nc.tensor.matmul(
                        psp,
                        attT[:, dc * N + i * P: dc * N + (i + 1) * P],
                        wp_sb[:, dc * C + co * 512: dc * C + co * 512 + 512],
                        start=(dc == 0), stop=(dc == PAIRS - 1))
                st = stage_pool.tile([P, 512], F16, tag="st")
                if evict_vector:
                    nc.vector.tensor_copy(st, psp)
                else:
                    nc.scalar.copy(st, psp)
                nc.sync.dma_start(
                    out=out_d[i * P:(i + 1) * P, co * 512: co * 512 + 512],
                    in_=st)

        # ---- boot PE work: K^T(p0,b0), Q^T(p0,b0), V(pairs 0-1, m 0-3) ----
        qt_tiles[0] = qt_pool.tile([P, N], F16, tag="qt", name="qt0")
        kt_tiles[0] = kt_pool.tile([P, N], F16, tag="kt", name="kt0")
        with tc.tile_pool(name="ps_boot", bufs=4, space="PSUM") as ps_boot:
            emit_qk_block(0, 1, 0)
            emit_qk_block(0, 0, 0)
            for m in range(4):
                emit_v_tile(0, m, ps_boot, evict_vector=False)

        ps_s = ctx.enter_context(tc.tile_pool(name="ps_s", bufs=2, space="PSUM"))
        ps_av = ctx.enter_context(tc.tile_pool(name="ps_av", bufs=1, space="PSUM"))

        # ---- fill units: small chunks of PE work dropped into the slack
        # of the exp-bound attention blocks ----
        def qtf(p, nb):
            return lambda: emit_qk_block(p, 0, nb)

        def ktf(p, nb):
            return lambda: emit_qk_block(p, 1, nb)

        def vf01(m):
            return lambda: emit_v_tile(0, m, ps_mm, evict_vector=True)

        def vf23(m):
            return lambda: emit_v_tile(1, m, ps_mm, evict_vector=True)

        def projf(i):
            return lambda: emit_proj(i, evict_vector=True)

        def newpair(p):
            def f():
                qt_tiles[p] = qt_pool.tile([P, N], F16, tag="qt", name=f"qt{p}")
                kt_tiles[p] = kt_pool.tile([P, N], F16, tag="kt", name=f"kt{p}")
                emit_wqk_dma(p)
            return f

        def misc1():  # wv(pairs 2-3) + pair-1 weights
            def f():
                emit_wv_dma(1)
                newpair(1)()
            return f

        def misc2():  # pair-3 weights + W_proj
            def f():
                newpair(3)()
                for dc in range(PAIRS):
                    nc.sync.dma_start(out=wp_sb[:, dc * C:(dc + 1) * C],
                                      in_=wp_d[dc * P:(dc + 1) * P, :])
            return f

        FILLS = {
            (0, 0): [ktf(0, 1), vf01(4), vf01(5), vf01(6),
                     ktf(0, 2), vf01(7), vf01(8), vf01(9),
                     ktf(0, 3), vf01(10), vf01(11), qtf(0, 1),
                     vf01(12), vf01(13), vf01(14), vf01(15)],
            (0, 1): [misc1(), qtf(0, 2)],
            (0, 2): [qtf(0, 3), vf23(0), vf23(1), vf23(2)],
            (0, 3): [ktf(1, 0), ktf(1, 1), qtf(1, 0), vf23(3), vf23(4)],
            (1, 0): [ktf(1, 2), ktf(1, 3), qtf(1, 1), vf23(5), vf23(6)],
            (1, 1): [newpair(2), qtf(1, 2), vf23(7), vf23(8)],
            (1, 2): [qtf(1, 3), ktf(2, 0), ktf(2, 1), vf23(9), vf23(10)],
            (1, 3): [ktf(2, 2), ktf(2, 3), qtf(2, 0), vf23(11), vf23(12)],
            (2, 0): [misc2(), qtf(2, 1), vf23(13), vf23(14), vf23(15)],
            (2, 1): [qtf(2, 2), ktf(3, 0), ktf(3, 1)],
            (2, 2): [qtf(2, 3), ktf(3, 2), ktf(3, 3)],
            (2, 3): [qtf(3, 0), qtf(3, 1), qtf(3, 2)],
            (3, 0): [qtf(3, 3)],
            (3, 1): [projf(0), projf(1), projf(2), projf(3)],
            (3, 2): [projf(4), projf(5), projf(6), projf(7)],
            (3, 3): [projf(8), projf(9), projf(10), projf(11)],
        }
        DENSE_FILL_AT = tuple(range(16))
        FILL_AT = (2, 5, 8, 11, 14)

        def emit_c_block(p, nb):
            qt_t, kt_t = qt_tiles[p], kt_tiles[p]
            fills = list(FILLS.get((p, nb), []))
            fill_at = DENSE_FILL_AT if (p, nb) == (0, 0) else FILL_AT
            nsl = slice(nb * 512, nb * 512 + 512)
            osl = slice(p * N + nb * 512, p * N + nb * 512 + 512)
            ps_av_t = ps_av.tile([P, 1024], F32, tag="av")
            for m in range(NT):
                if m in fill_at and fills:
                    fills.pop(0)()
                first, last = (m == 0), (m == NT - 1)
                ps_s_t = ps_s.tile([P, 1024], F32, tag="s")
                nc.tensor.matmul(
                    ps_s_t[:, 0:512],
                    kt_t[0:64, m * P:(m + 1) * P], qt_t[0:64, nsl],
                    start=True, stop=True)
                nc.tensor.matmul(
                    ps_s_t[:, 512:1024],
                    kt_t[64:128, m * P:(m + 1) * P], qt_t[64:128, nsl],
                    start=True, stop=True)
                pt = pt_pool.tile([P, 1024], F16, tag="pt")
                nc.scalar.activation(pt, ps_s_t, AF.Exp,
                                     scale=SCALE, bias=ebias)
                vbase = m * MBLK + 2 * p * VW
                nc.tensor.matmul(
                    ps_av_t[0:VW, 0:512],
                    v_sb[:, vbase: vbase + VW],
                    pt[:, 0:512],
                    start=first, stop=last, skip_group_check=True)
                nc.tensor.matmul(
                    ps_av_t[0:VW, 512:1024],
                    v_sb[:, vbase + VW: vbase + 2 * VW],
                    pt[:, 512:1024],
                    start=first, stop=last, skip_group_check=True)
            while fills:
                fills.pop(0)()
            # evict: head A rows -> attT, head B rows -> tmb, sums -> DRAM
            nc.vector.tensor_copy(attT[0:64, osl], ps_av_t[0:64, 0:512])
            tmb = tmb_pool.tile([64, 512], F16, tag="tmb")
            nc.vector.tensor_copy(tmb, ps_av_t[0:64, 512:1024])
            srow = sums_pool.tile([P, 1024], F32, tag="srow")
            nc.vector.tensor_copy(srow[64:65, :], ps_av_t[64:65, :])
            dsum = dram_pool.tile([1024], F32, tag="dsum")
            nc.gpsimd.dma_start(out=dsum.rearrange("(a b) -> a b", a=1),
                                in_=srow[64:65, :])
            # normalization (PE-free): spread sums across 128 partitions via
            # DRAM, wide reciprocal, stride-0 broadcast back, multiply in.
            spread = spread_pool.tile([P, 8], F32, tag="spf")
            nc.gpsimd.dma_start(out=spread,
                                in_=dsum.rearrange("(q f) -> q f", q=P))
            spreadr = spread_pool.tile([P, 8], F16, tag="sph")
            with nc.allow_low_precision(reason="softmax recip rounding"):
                nc.vector.reciprocal(spreadr, spread)
            drec = dram_pool.tile([1024], F16, tag="drec")
            nc.gpsimd.dma_start(out=drec.rearrange("(q f) -> q f", q=P),
                                in_=spreadr)
            rb = rb_pool.tile([64, 1024], F16, tag="rb")
            nc.gpsimd.dma_start(
                out=rb,
                in_=drec.rearrange("(a b) -> a b", a=1).broadcast_to([64, 1024]))
            nc.vector.tensor_mul(attT[0:64, osl], attT[0:64, osl], rb[:, 0:512])
            nc.vector.tensor_mul(tmb, tmb, rb[:, 512:1024])
            nc.sync.dma_start(out=attT[64:128, osl], in_=tmb)

        for p in range(PAIRS):
            for nb in range(NB):
                emit_c_block(p, nb)

        # ---- tail: projection of the last query block ----
        for i in range(4 * (NB - 1), 4 * NB):
            emit_proj(i, evict_vector=False)


@functools.lru_cache(maxsize=1)
def build_nc():
    nc = bacc.Bacc("TRN2", target_bir_lowering=False, debug=False)
    xtb_d = nc.dram_tensor("xt_blocks", [NB * CT * P, 512], F16,
                           kind="ExternalInput").ap()
    wq_d = nc.dram_tensor("wq", [C, DCORE], F16, kind="ExternalInput").ap()
    wk_d = nc.dram_tensor("wk", [C, DCORE], F16, kind="ExternalInput").ap()
    wv_d = nc.dram_tensor("wv", [C, DCORE], F16, kind="ExternalInput").ap()
    wp_d = nc.dram_tensor("wp", [DCORE, C], F16, kind="ExternalInput").ap()
    out_d = nc.dram_tensor("out_partial", [N, C], F16, kind="ExternalOutput").ap()
    with tile.TileContext(nc) as tc:
        _kernel_body(tc, out_d, xtb_d, wq_d, wk_d, wv_d, wp_d)
    nc.compile()
    return nc


def make_in_maps(x, W_qkv, W_proj):
    in_maps = []
    for core in range(NCORES):
        b, half = core // 2, core % 2
        h0 = half * HPC
        xt = x[b].T.astype(np.float16)              # [C, N]
        xtb = np.ascontiguousarray(
            xt.reshape(CT, P, NB, 512).transpose(2, 0, 1, 3)
        ).reshape(NB * CT * P, 512)
        in_maps.append({
            "xt_blocks": xtb,
            "wq": np.ascontiguousarray(
                W_qkv[:, 0 * C + h0 * D: 0 * C + h0 * D + DCORE].astype(np.float16)),
            "wk": np.ascontiguousarray(
                W_qkv[:, 1 * C + h0 * D: 1 * C + h0 * D + DCORE].astype(np.float16)),
            "wv": np.ascontiguousarray(
                W_qkv[:, 2 * C + h0 * D: 2 * C + h0 * D + DCORE].astype(np.float16)),
            "wp": np.ascontiguousarray(
                W_proj[h0 * D: h0 * D + DCORE, :].astype(np.float16)),
        })
    return in_maps


def kernel(x, W_qkv, W_proj, b_proj, trace=False):
    x = np.asarray(x, dtype=np.float32)
    W_qkv = np.asarray(W_qkv, dtype=np.float32)
    W_proj = np.asarray(W_proj, dtype=np.float32)
    b_proj = np.asarray(b_proj, dtype=np.float32)

    nc = build_nc()
    in_maps = make_in_maps(x, W_qkv, W_proj)

    global LAST_RESULT
    res = run_bass_kernel_spmd(nc, in_maps, list(range(NCORES)), trace=trace)
    LAST_RESULT = res

    out = np.empty((B, N, C), dtype=np.float32)
    for b in range(B):
        out[b] = (res.results[2 * b]["out_partial"].astype(np.float32)
                  + res.results[2 * b + 1]["out_partial"].astype(np.float32)
                  + b_proj[None, :])
    return out


# revision 9
# speedup vs baseline: 1.4391x; 1.0114x over previous
"""Multi-head attention (B=4, N=2048, C=1024, H=16) on 8 TRN2 NeuronCores.

Sharding: core = 2*b + half handles batch b, heads half*8 .. half*8+7.
Each core computes QKV for its 8 heads, full attention for them, and a
partial projection (its 512 rows of W_proj). Host sums the two partials
per batch and adds the bias.

v3 schedule: the scalar engine's exp stream (256 x [128,1024]) is the
critical resource; everything else is arranged so neither it nor the
PE ever hits a head-of-line block:
  - x^T is staged in DRAM as 32 contiguous (query-block, c-chunk)
    blocks and DMA'd in dependency-chained groups, so K^T/Q^T for the
    first query block (and the exp stream) start after ~1 MB of
    traffic instead of after the full 4 MB.
  - softmax normalization is PE-free and off the critical path: the
    denominators ride the AV matmuls as a 65th stationary column; the
    sums row is evicted, spread across 128 partitions via a DRAM
    round-trip, reciprocal'd wide (~0.2us instead of 6.5us on one
    partition), broadcast back with a stride-0 DMA and multiplied in
    on DVE.  These DMAs ride the gpsimd (SWDGE) queue so the sync
    queue never blocks on them.
  - V (pairs 0-1 beyond the first tiles), V (pairs 2-3), Q/K for later
    pairs and the first 12 projection tiles are emitted as small fill
    units inside the attention blocks, sized to the PE slack there.
  - PSUM evictions go to the scalar engine only where it is idle
    (boot, projection tail), otherwise to DVE.

All matmul operands are fp16 (1 cycle/row on the PE), accumulation
fp32 in PSUM. Host pre-casts weights/x and pre-transposes x; output
partials return as fp16 and are summed on the host in fp32.
"""

import functools
from contextlib import ExitStack

import numpy as np

import concourse.bass as bass
import concourse.tile as tile
from concourse.tile import add_dep_helper
from concourse import bacc, mybir
from concourse.bass_utils import run_bass_kernel_spmd

F32 = mybir.dt.float32
F16 = mybir.dt.float16
AF = mybir.ActivationFunctionType

B, N, C = 4, 2048, 1024
H, D = 16, 64
P = 128
NCORES = 8
HPC = 8            # heads per core
PAIRS = HPC // 2   # 4
DCORE = HPC * D    # 512 attention columns per core
SCALE = float(H) ** -0.5  # 0.25 (faithful to reference: num_heads**-0.5)
EXP_BIAS = -5.0    # exp(scale*s + bias): cancels in softmax, keeps fp16 range
NB = N // 512      # 4 query blocks
NT = N // P        # 16 key tiles of 128
CT = C // P        # 8 contraction chunks
VW = D + 1         # V columns per head incl. the ones column (row sums)
MBLK = HPC * VW    # 520 v_sb columns per m-tile
HB = DCORE // 2    # 256 V columns per half (head pairs 0-1 / 2-3)

LAST_RESULT = None  # BassKernelResults of the most recent run (for test.py)


def _kernel_body(tc, out_d, xtb_d, wq_d, wk_d, wv_d, wp_d):
    nc = tc.nc
    with ExitStack() as ctx:
        const = ctx.enter_context(tc.tile_pool(name="const", bufs=1))
        ones_f = const.tile([P, P], F32)
        nc.vector.memset(ones_f, 1.0)
        ebias = const.tile([P, 1], F32)
        nc.vector.memset(ebias, EXP_BIAS)

        # attT: pair p occupies cols [p*N, (p+1)*N); partitions = 2 heads x 64
        attT_pool = ctx.enter_context(tc.tile_pool(name="attT", bufs=1))
        attT = attT_pool.tile([P, PAIRS * N], F16)
        xt_pool = ctx.enter_context(tc.tile_pool(name="xt", bufs=1))
        xt = xt_pool.tile([P, CT * N], F16)
        v_pool = ctx.enter_context(tc.tile_pool(name="v", bufs=1))
        v_sb = v_pool.tile([P, NT * MBLK], F16)
        wv_pool = ctx.enter_context(tc.tile_pool(name="wv", bufs=1))
        wv_sb = wv_pool.tile([P, CT * DCORE], F16)
        wp_pool = ctx.enter_context(tc.tile_pool(name="wp", bufs=1))
        wp_sb = wp_pool.tile([P, PAIRS * C], F16)

        qt_pool = ctx.enter_context(tc.tile_pool(name="qt", bufs=2))
        kt_pool = ctx.enter_context(tc.tile_pool(name="kt", bufs=2))
        wqk_pool = ctx.enter_context(tc.tile_pool(name="wqk", bufs=4))
        pt_pool = ctx.enter_context(tc.tile_pool(name="pt", bufs=6))
        tmb_pool = ctx.enter_context(tc.tile_pool(name="tmb", bufs=3))
        sums_pool = ctx.enter_context(tc.tile_pool(name="sums", bufs=2))
        spread_pool = ctx.enter_context(tc.tile_pool(name="spread", bufs=2))
        rb_pool = ctx.enter_context(tc.tile_pool(name="rb", bufs=2))
        stage_pool = ctx.enter_context(tc.tile_pool(name="stage", bufs=3))
        dram_pool = ctx.enter_context(
            tc.tile_pool(name="dscr", bufs=3, space="DRAM"))

        ps_mm = ctx.enter_context(tc.tile_pool(name="ps_mm", bufs=2, space="PSUM"))

        qt_tiles = [None] * PAIRS
        kt_tiles = [None] * PAIRS
        wt_tiles = [None] * PAIRS

        def emit_wqk_dma(p):
            tiles = []
            for w_d in (wq_d, wk_d):
                wt = wqk_pool.tile([P, CT * P], F16, tag="w")
                nc.sync.dma_start(
                    out=wt.rearrange("q (cc f) -> q cc f", cc=CT),
                    in_=w_d[:, p * P:(p + 1) * P].rearrange(
                        "(cc q) f -> q cc f", q=P))
                tiles.append(wt)
            wt_tiles[p] = tiles

        def emit_wv_dma(half):
            c0 = half * HB
            insts = []
            for cc in range(CT):
                insts.append(nc.sync.dma_start(
                    out=wv_sb[:, cc * DCORE + c0: cc * DCORE + c0 + HB],
                    in_=wv_d[cc * P:(cc + 1) * P, c0:c0 + HB]))
            return insts

        # ---- boot DMAs, dependency-chained so the first query block's
        # x^T (1 MB) and wv(pairs 0-1) land before the rest of x^T ----
        emit_wqk_dma(0)
        xt_last = {}

        def emit_xt_group(nb, after=None):
            last = None
            for cc in range(CT):
                ins = nc.sync.dma_start(
                    out=xt[:, cc * N + nb * 512: cc * N + nb * 512 + 512],
                    in_=xtb_d[(nb * CT + cc) * P:(nb * CT + cc + 1) * P, :])
                if after is not None:
                    add_dep_helper(ins.ins, after.ins, sync=True,
                                   reason="boot DMA pacing")
                last = ins
            xt_last[nb] = last
            return last

        g_a = emit_xt_group(0)
        wv01 = emit_wv_dma(0)
        for ins in wv01:
            add_dep_helper(ins.ins, g_a.ins, sync=True,
                           reason="boot DMA pacing")
        g_c = emit_xt_group(1, after=wv01[-1])
        g_d = emit_xt_group(2, after=g_c)
        emit_xt_group(3, after=g_d)

        # ones columns of v_sb (fused softmax row sums)
        ones_cols = v_sb.rearrange("q (g k) -> q g k", k=VW)[:, :, D:VW]
        nc.vector.tensor_copy(
            ones_cols, ones_f.rearrange("q (g k) -> q g k", k=1))

        def emit_v_tile(half, m, evict_vector):
            c0 = half * HB
            psv = ps_mm.tile([P, HB], F32, tag="mm")
            for cc in range(CT):
                nc.tensor.matmul(
                    psv,
                    xt[:, cc * N + m * P: cc * N + (m + 1) * P],
                    wv_sb[:, cc * DCORE + c0: cc * DCORE + c0 + HB],
                    start=(cc == 0), stop=(cc == CT - 1))
            base = m * MBLK + 4 * half * VW
            dst = v_sb[:, base: base + 4 * VW].rearrange(
                "q (h k) -> q h k", k=VW)[:, :, 0:D]
            src = psv.rearrange("q (h k) -> q h k", k=D)
            if evict_vector:
                nc.vector.tensor_copy(dst, src)
            else:
                nc.scalar.copy(dst, src)

        def emit_qk_block(p, which, nb):
            # which: 0 = q, 1 = k
            wt = wt_tiles[p][which]
            dst = (qt_tiles if which == 0 else kt_tiles)[p]
            psq = ps_mm.tile([P, 512], F32, tag="mm")
            for cc in range(CT):
                nc.tensor.matmul(
                    psq,
                    wt[:, cc * P:(cc + 1) * P],
                    xt[:, cc * N + nb * 512: cc * N + nb * 512 + 512],
                    start=(cc == 0), stop=(cc == CT - 1))
            nc.vector.tensor_copy(dst[:, nb * 512:(nb + 1) * 512], psq)

        def emit_proj(i, evict_vector):
            for co in range(2):
                psp = ps_mm.tile([P, 512], F32, tag="mm")
                for dc in range(PAIRS):
                    nc.tensor.matmul(
                        psp,
                        attT[:, dc * N + i * P: dc * N + (i + 1) * P],
                        wp_sb[:, dc * C + co * 512: dc * C + co * 512 + 512],
                        start=(dc == 0), stop=(dc == PAIRS - 1))
                st = stage_pool.tile([P, 512], F16, tag="st")
                if evict_vector:
                    nc.vector.tensor_copy(st, psp)
                else:
                    nc.scalar.copy(st, psp)
                nc.sync.dma_start(
                    out=out_d[i * P:(i + 1) * P, co * 512: co * 512 + 512],
                    in_=st)

        # ---- boot PE work: K^T(p0,b0), Q^T(p0,b0), V(pairs 0-1, m 0-3) ----
        qt_tiles[0] = qt_pool.tile([P, N], F16, tag="qt", name="qt0")
        kt_tiles[0] = kt_pool.tile([P, N], F16, tag="kt", name="kt0")
        emit_qk_block(0, 1, 0)
        emit_qk_block(0, 0, 0)
        emit_v_tile(0, 0, evict_vector=True)
        emit_v_tile(0, 1, evict_vector=True)

        ps_s = ctx.enter_context(tc.tile_pool(name="ps_s", bufs=2, space="PSUM"))
        ps_av = ctx.enter_context(tc.tile_pool(name="ps_av", bufs=1, space="PSUM"))

        # ---- fill units: small chunks of PE work dropped into the slack
        # of the exp-bound attention blocks ----
        def qtf(p, nb):
            return lambda: emit_qk_block(p, 0, nb)

        def ktf(p, nb):
            return lambda: emit_qk_block(p, 1, nb)

        def vf01(m):
            return lambda: emit_v_tile(0, m, evict_vector=True)

        def vf23(m):
            return lambda: emit_v_tile(1, m, evict_vector=True)

        def projf(i):
            return lambda: emit_proj(i, evict_vector=True)

        def newpair(p):
            def f():
                qt_tiles[p] = qt_pool.tile([P, N], F16, tag="qt", name=f"qt{p}")
                kt_tiles[p] = kt_pool.tile([P, N], F16, tag="kt", name=f"kt{p}")
                emit_wqk_dma(p)
            return f

        def misc1():  # wv(pairs 2-3) + pair-1 weights
            def f():
                emit_wv_dma(1)
                newpair(1)()
            return f

        def misc2():  # pair-3 weights + W_proj
            def f():
                newpair(3)()
                for dc in range(PAIRS):
                    nc.sync.dma_start(out=wp_sb[:, dc * C:(dc + 1) * C],
                                      in_=wp_d[dc * P:(dc + 1) * P, :])
            return f

        FILLS = {
            (0, 0): [vf01(2), vf01(3), ktf(0, 1), vf01(4), vf01(5),
                     vf01(6), ktf(0, 2), vf01(7), vf01(8), ktf(0, 3),
                     vf01(9), vf01(10), qtf(0, 1), vf01(11), vf01(12),
                     vf01(13), vf01(14), vf01(15)],
            (0, 1): [misc1(), qtf(0, 2)],
            (0, 2): [qtf(0, 3), vf23(0), vf23(1), vf23(2)],
            (0, 3): [ktf(1, 0), ktf(1, 1), qtf(1, 0), vf23(3), vf23(4)],
            (1, 0): [ktf(1, 2), ktf(1, 3), qtf(1, 1), vf23(5), vf23(6)],
            (1, 1): [newpair(2), qtf(1, 2), vf23(7), vf23(8)],
            (1, 2): [qtf(1, 3), ktf(2, 0), ktf(2, 1), vf23(9), vf23(10)],
            (1, 3): [ktf(2, 2), ktf(2, 3), qtf(2, 0), vf23(11), vf23(12)],
            (2, 0): [misc2(), qtf(2, 1), vf23(13), vf23(14), vf23(15)],
            (2, 1): [qtf(2, 2), ktf(3, 0), ktf(3, 1)],
            (2, 2): [qtf(2, 3), ktf(3, 2), ktf(3, 3)],
            (2, 3): [qtf(3, 0), qtf(3, 1), qtf(3, 2)],
            (3, 0): [qtf(3, 3)],
            (3, 1): [projf(0), projf(1), projf(2), projf(3)],
            (3, 2): [projf(4), projf(5), projf(6), projf(7)],
            (3, 3): [projf(8), projf(9), projf(10), projf(11)],
        }
        # fills consumed per m-slot: (0,0) takes 2 per slot early (it is
        # DMA-paced anyway); other blocks 1 at each of 5 points.
        DENSE_FILL_CNT = {m: (2 if m < 8 else 1) for m in range(16)}
        FILL_CNT = {2: 1, 5: 1, 8: 1, 11: 1, 14: 1}

        def emit_c_block(p, nb):
            qt_t, kt_t = qt_tiles[p], kt_tiles[p]
            fills = list(FILLS.get((p, nb), []))
            fill_cnt = DENSE_FILL_CNT if (p, nb) == (0, 0) else FILL_CNT
            nsl = slice(nb * 512, nb * 512 + 512)
            osl = slice(p * N + nb * 512, p * N + nb * 512 + 512)
            ps_av_t = ps_av.tile([P, 1024], F32, tag="av")
            for m in range(NT):
                for _ in range(fill_cnt.get(m, 0)):
                    if fills:
                        fills.pop(0)()
                first, last = (m == 0), (m == NT - 1)
                ps_s_t = ps_s.tile([P, 1024], F32, tag="s")
                nc.tensor.matmul(
                    ps_s_t[:, 0:512],
                    kt_t[0:64, m * P:(m + 1) * P], qt_t[0:64, nsl],
                    start=True, stop=True)
                nc.tensor.matmul(
                    ps_s_t[:, 512:1024],
                    kt_t[64:128, m * P:(m + 1) * P], qt_t[64:128, nsl],
                    start=True, stop=True)
                pt = pt_pool.tile([P, 1024], F16, tag="pt")
                nc.scalar.activation(pt, ps_s_t, AF.Exp,
                                     scale=SCALE, bias=ebias)
                vbase = m * MBLK + 2 * p * VW
                nc.tensor.matmul(
                    ps_av_t[0:VW, 0:512],
                    v_sb[:, vbase: vbase + VW],
                    pt[:, 0:512],
                    start=first, stop=last, skip_group_check=True)
                nc.tensor.matmul(
                    ps_av_t[0:VW, 512:1024],
                    v_sb[:, vbase + VW: vbase + 2 * VW],
                    pt[:, 512:1024],
                    start=first, stop=last, skip_group_check=True)
            while fills:
                fills.pop(0)()
            # evict bank 0 first (head A + its sums), then bank 1, so the
            # next block's AV start on bank 0 unblocks as early as possible
            dmae = nc.sync if p == PAIRS - 1 else nc.gpsimd
            nc.vector.tensor_copy(attT[0:64, osl], ps_av_t[0:64, 0:512])
            srow = sums_pool.tile([P, 1024], F32, tag="srow")
            nc.vector.tensor_copy(srow[64:65, 0:512], ps_av_t[64:65, 0:512])
            tmb = tmb_pool.tile([64, 512], F16, tag="tmb")
            nc.vector.tensor_copy(tmb, ps_av_t[0:64, 512:1024])
            nc.vector.tensor_copy(srow[64:65, 512:1024],
                                  ps_av_t[64:65, 512:1024])
            dsum = dram_pool.tile([1024], F32, tag="dsum")
            dmae.dma_start(out=dsum.rearrange("(a b) -> a b", a=1),
                           in_=srow[64:65, :])
            # normalization (PE-free): spread sums across 128 partitions via
            # DRAM, wide reciprocal, stride-0 broadcast back, multiply in.
            spread = spread_pool.tile([P, 8], F32, tag="spf")
            dmae.dma_start(out=spread,
                           in_=dsum.rearrange("(q f) -> q f", q=P))
            spreadr = spread_pool.tile([P, 8], F16, tag="sph")
            with nc.allow_low_precision(reason="softmax recip rounding"):
                nc.vector.reciprocal(spreadr, spread)
            drec = dram_pool.tile([1024], F16, tag="drec")
            dmae.dma_start(out=drec.rearrange("(q f) -> q f", q=P),
                           in_=spreadr)
            rb = rb_pool.tile([64, 1024], F16, tag="rb")
            dmae.dma_start(
                out=rb,
                in_=drec.rearrange("(a b) -> a b", a=1).broadcast_to([64, 1024]))
            nc.vector.tensor_mul(attT[0:64, osl], attT[0:64, osl], rb[:, 0:512])
            nc.vector.tensor_mul(tmb, tmb, rb[:, 512:1024])
            nc.sync.dma_start(out=attT[64:128, osl], in_=tmb)

        for p in range(PAIRS):
            for nb in range(NB):
                emit_c_block(p, nb)

        # ---- tail: projection of the last query block ----
        for i in range(4 * (NB - 1), 4 * NB):
            emit_proj(i, evict_vector=False)


@functools.lru_cache(maxsize=1)
def build_nc():
    nc = bacc.Bacc("TRN2", target_bir_lowering=False, debug=False)
    xtb_d = nc.dram_tensor("xt_blocks", [NB * CT * P, 512], F16,
                           kind="ExternalInput").ap()
    wq_d = nc.dram_tensor("wq", [C, DCORE], F16, kind="ExternalInput").ap()
    wk_d = nc.dram_tensor("wk", [C, DCORE], F16, kind="ExternalInput").ap()
    wv_d = nc.dram_tensor("wv", [C, DCORE], F16, kind="ExternalInput").ap()
    wp_d = nc.dram_tensor("wp", [DCORE, C], F16, kind="ExternalInput").ap()
    out_d = nc.dram_tensor("out_partial", [N, C], F16, kind="ExternalOutput").ap()
    with tile.TileContext(nc) as tc:
        _kernel_body(tc, out_d, xtb_d, wq_d, wk_d, wv_d, wp_d)
    nc.compile()
    return nc


def make_in_maps(x, W_qkv, W_proj):
    in_maps = []
    for core in range(NCORES):
        b, half = core // 2, core % 2
        h0 = half * HPC
        xt = x[b].T.astype(np.float16)              # [C, N]
        xtb = np.ascontiguousarray(
            xt.reshape(CT, P, NB, 512).transpose(2, 0, 1, 3)
        ).reshape(NB * CT * P, 512)
        in_maps.append({
            "xt_blocks": xtb,
            "wq": np.ascontiguousarray(
                W_qkv[:, 0 * C + h0 * D: 0 * C + h0 * D + DCORE].astype(np.float16)),
            "wk": np.ascontiguousarray(
                W_qkv[:, 1 * C + h0 * D: 1 * C + h0 * D + DCORE].astype(np.float16)),
            "wv": np.ascontiguousarray(
                W_qkv[:, 2 * C + h0 * D: 2 * C + h0 * D + DCORE].astype(np.float16)),
            "wp": np.ascontiguousarray(
                W_proj[h0 * D: h0 * D + DCORE, :].astype(np.float16)),
        })
    return in_maps


def kernel(x, W_qkv, W_proj, b_proj, trace=False):
    x = np.asarray(x, dtype=np.float32)
    W_qkv = np.asarray(W_qkv, dtype=np.float32)
    W_proj = np.asarray(W_proj, dtype=np.float32)
    b_proj = np.asarray(b_proj, dtype=np.float32)

    nc = build_nc()
    in_maps = make_in_maps(x, W_qkv, W_proj)

    global LAST_RESULT
    res = run_bass_kernel_spmd(nc, in_maps, list(range(NCORES)), trace=trace)
    LAST_RESULT = res

    out = np.empty((B, N, C), dtype=np.float32)
    for b in range(B):
        out[b] = (res.results[2 * b]["out_partial"].astype(np.float32)
                  + res.results[2 * b + 1]["out_partial"].astype(np.float32)
                  + b_proj[None, :])
    return out
